# revision 70
# baseline (speedup 1.0000x reference)
"""LoRA Linear (residual + low-rank path with dropout) on 8 Trainium2 cores.

Math (fp32 reference):
  residual = hidden_states @ W_base.T
  dropped  = hidden_states * dropout_mask / (1 - p)
  out      = residual + ((dropped @ A.T) @ B.T) * scaling

Sharding: data-parallel over the 8192 tokens (8 cores x 1024 tokens);
W_base / A / B replicated. Inputs are cast to bf16 on the host (PSUM
accumulation stays fp32): same 1 cycle/row PE rate as f32r but half
the DMA bytes and SBUF, and bf16 weights get Fast-Weight-Load (4x
faster LDWEIGHTS, which fp32 is excluded from). Error ~3e-3 rel-rms /
~4e-3 scale-relative absmax over the K=4096 contraction, an order of
magnitude inside the 2e-2 gate.

Layout: W is the STATIONARY operand ([128 d, 128 o] chunks) and x is
the MOVING operand (512 tokens/matmul = one fp32 PSUM bank). One
(FWL) weight load serves the 1024-row matmul pair, so the steady
state issues back-to-back 512-row matmuls at the ~216ns hardware
floor (512 cycles @ 2.4 GHz + NX dispatch).

Schedule: the x+mask load is the serial head, so the k-loop that
computes the LoRA xa product also carries the main-matmul
accumulation for the first P_OC=3 out-chunks (6 PSUM banks + 2 xa
banks = all 8). The remaining 29 chunks then run back-to-back with W
(1 MiB/chunk) double-buffered against compute.

  - W streams exactly once (x stays resident in SBUF); host pre-tiles
    everything into large contiguous DMA runs. The P_OC prologue W
    chunks ship as one merged tensor (W012): sync-engine DMA-issue
    slots (~1.4us each), not bytes, are the scarce prologue resource,
    so x/W ride the sync ring and mask/AT ride the scalar ring.
  - Output DMAs issue from the ACT engine so the SP engine's HWDGE
    stream (all input loads) never blocks on a compute semaphore.
  - The rank-16 LoRA product accumulates into the same PSUM tile as
    the residual matmul (K=16 matmul, start=False), so the add is free.
  - 1/(1-p) is folded into A, `scaling` into B on the host.
  - Output leaves the device transposed ([out_chunk, out_lane, token]);
    the host gather undoes it.
"""

import numpy as np

P = 128
D_IN = 4096
D_OUT = 4096
BATCH, SEQ = 4, 2048
TOK = BATCH * SEQ  # 8192
NCORES = 8
T = TOK // NCORES  # 1024 tokens per core, all resident
KT = D_IN // P  # 32 k-tiles
OB = 128  # out-dim chunk width (stationary operand)
OCB = D_OUT // OB  # 32 out chunks
NT = 512  # moving free dim (tokens per matmul) = fp32 max = 1 PSUM bank
TH = T // NT  # 2 token halves
R = 16
PIECE = 4  # k-tiles per steady W DMA piece
# small leading pieces for an early PE start, coarse later blocks to
# keep the sync engine's per-DMA issue cost (~1.4us) off the critical path
BLOCKS = [(0, 1), (1, 2), (2, 4)] + [(k, k + 4) for k in range(4, KT, 4)]
P_OC = 3  # out-chunks folded into the prologue k-loop
WPRE = 1  # W prefetch depth (chunks ahead) in the steady loop
DROP_P = 0.05
SCALING = 32.0 / 16.0

_PROGRAM_CACHE = {}


def _build_program():
    from concourse import bacc
    import concourse.mybir as mybir
    import concourse.tile as tile

    f32 = mybir.dt.float32
    bf16 = mybir.dt.bfloat16
    u8 = mybir.dt.uint8

    nc = bacc.Bacc("TRN2", target_bir_lowering=False)
    xT_d = nc.dram_tensor("xT", [KT, P, T], bf16, kind="ExternalInput")
    mT_d = nc.dram_tensor("mT", [KT, P, T], bf16, kind="ExternalInput")
    WT_d = nc.dram_tensor("WT", [OCB, KT, P, OB], bf16, kind="ExternalInput")
    W012_d = nc.dram_tensor("W012", [KT, P, P_OC * OB], bf16, kind="ExternalInput")
    AT_d = nc.dram_tensor("AT", [P, KT, R], bf16, kind="ExternalInput")
    BT_d = nc.dram_tensor("BT", [R, D_OUT], bf16, kind="ExternalInput")
    out_d = nc.dram_tensor("out", [OCB, P, T], f32, kind="ExternalOutput")

    with tile.TileContext(nc) as tc:
        with (
            tc.tile_pool(name="xt", bufs=1) as xtpool,
            tc.tile_pool(name="at", bufs=1) as atpool,
            tc.tile_pool(name="bt", bufs=4) as btpool,
            tc.tile_pool(name="wt", bufs=4) as wtpool,
            tc.tile_pool(name="w012", bufs=1) as w012pool,
            tc.tile_pool(name="m", bufs=2) as mpool,
            tc.tile_pool(name="d", bufs=3) as dpool,
            tc.tile_pool(name="xa", bufs=1) as xapool,
            tc.tile_pool(name="o", bufs=4) as opool,
            tc.tile_pool(name="ps_mm", bufs=8, space="PSUM") as ps_mm,
        ):
            xT_t = xtpool.tile([P, KT, T], bf16, tag="xT")
            at_t = atpool.tile([P, KT, R], bf16, tag="AT")
            wt = {}
            bt = {}

            def load_bt(oc):
                bt[oc] = btpool.tile([R, OB], bf16, tag="BT", name=f"BT{oc}")
                nc.sync.dma_start(bt[oc][:], BT_d[:, oc * OB : (oc + 1) * OB])

            def new_wt(oc):
                wt[oc] = wtpool.tile([P, KT, OB], bf16, tag="WT", name=f"WT{oc}")

            def load_wt_piece(oc, k0, n=PIECE):
                nc.sync.dma_start(
                    wt[oc][:, k0 : k0 + n],
                    WT_d[oc, k0 : k0 + n].rearrange("k p o -> p k o"),
                )

            def load_wt(oc):
                new_wt(oc)
                for k0 in range(0, KT, 2 * PIECE):
                    load_wt_piece(oc, k0, 2 * PIECE)

            # the P_OC prologue W chunks share one tile so each k-block is a
            # single merged DMA (sync-engine issue slots are the scarce
            # resource in the prologue, not bytes)
            w012 = w012pool.tile([P, KT, P_OC, OB], bf16, tag="W012", name="W012t")
            for o in range(P_OC):
                load_bt(o)

            xa_ps = [
                ps_mm.tile([R, NT], f32, tag="ps", name=f"xa_ps{h}")
                for h in range(TH)
            ]
            pro_ps = {
                (o, h): ps_mm.tile([P, NT], f32, tag="ps", name=f"pps{o}_{h}")
                for o in range(P_OC)
                for h in range(TH)
            }

            # ---- prologue k-loop: x/mask stream in; xa (LoRA stage 1) and
            # the first P_OC out-chunks of the residual matmul accumulate.
            m_t = None
            mk0 = 0
            for kb, (k0, k1) in enumerate(BLOCKS):
                n = k1 - k0
                # sync engine: x then the merged W block; scalar engine
                # (idle until the first drain) issues mask/AT in parallel
                nc.sync.dma_start(
                    xT_t[:, k0:k1],
                    xT_d[k0:k1].rearrange("k p t -> p k t"),
                )
                nc.sync.dma_start(
                    w012[:, k0:k1],
                    W012_d[k0:k1].rearrange("k p b -> p k b"),
                )
                m_t = mpool.tile([P, n, T], bf16, tag="m", name=f"m{k0}")
                mk0 = k0
                nc.scalar.dma_start(
                    m_t[:], mT_d[k0:k1].rearrange("k p t -> p k t")
                )
                if kb == 0:
                    nc.scalar.dma_start(at_t[:], AT_d[:])
                for k in range(k0, k1):
                    # d-mults first (DVE overlaps the main matmuls), then
                    # mains with each stationary serving both halves, then
                    # the xa pair sharing the AT stationary.
                    dts = []
                    for h in range(TH):
                        hs = slice(h * NT, (h + 1) * NT)
                        d_t = dpool.tile([P, NT], bf16, tag="d", name=f"d{k}_{h}")
                        nc.vector.tensor_tensor(
                            d_t[:],
                            xT_t[:, k, hs],
                            m_t[:, k - mk0, hs],
                            mybir.AluOpType.mult,
                        )
                        dts.append(d_t)
                    for o in range(P_OC):
                        for h in range(TH):
                            hs = slice(h * NT, (h + 1) * NT)
                            nc.tensor.matmul(
                                pro_ps[o, h][:],
                                w012[:, k, o],
                                xT_t[:, k, hs],
                                start=(k == 0),
                                stop=False,
                            )
                    for h in range(TH):
                        nc.tensor.matmul(
                            xa_ps[h][:],
                            at_t[:, k],
                            dts[h][:],
                            start=(k == 0),
                            stop=(k == KT - 1),
                        )
            # W prefetch for the first steady chunk: the sync ring drains
            # ~20us before the PE finishes the prologue, so this lands early.
            load_wt(P_OC)

            xaT_t = xapool.tile([R, T], bf16, tag="xaT")
            for h in range(TH):
                nc.vector.tensor_copy(
                    xaT_t[:, h * NT : (h + 1) * NT], xa_ps[h][:]
                )

            def finish(oc, pss):
                # rank-16 LoRA accumulate + drain
                for h in range(TH):
                    hs = slice(h * NT, (h + 1) * NT)
                    nc.tensor.matmul(
                        pss[h][:],
                        bt[oc][:],
                        xaT_t[:, hs],
                        start=False,
                        stop=True,
                    )
                for h in range(TH):
                    hs = slice(h * NT, (h + 1) * NT)
                    o_t = opool.tile([P, NT], f32, tag="o", name=f"o{oc}_{h}")
                    nc.vector.tensor_copy(o_t[:], pss[h][:])
                    nc.scalar.dma_start(out_d[oc, :, hs], o_t[:])

            for o in range(P_OC):
                finish(o, [pro_ps[o, h] for h in range(TH)])

            # ---- steady loop over the remaining out-chunks
            for oc in range(P_OC, OCB):
                load_bt(oc)
                if oc + WPRE < OCB:
                    load_wt(oc + WPRE)
                pss = [
                    ps_mm.tile([P, NT], f32, tag="ps", name=f"ps{oc}_{h}")
                    for h in range(TH)
                ]
                for k in range(KT):
                    for h in range(TH):
                        nc.tensor.matmul(
                            pss[h][:],
                            wt[oc][:, k],
                            xT_t[:, k, h * NT : (h + 1) * NT],
                            start=(k == 0),
                            stop=False,
                        )
                finish(oc, pss)
                del wt[oc]

    nc.finalize()
    return nc


def _get_program():
    if "nc" not in _PROGRAM_CACHE:
        _PROGRAM_CACHE["nc"] = _build_program()
    return _PROGRAM_CACHE["nc"]


def kernel(hidden_states, W_base, A, B, dropout_mask):
    from concourse.bass_utils import run_bass_kernel_spmd

    hs = np.ascontiguousarray(np.asarray(hidden_states, dtype=np.float32)).reshape(
        TOK, D_IN
    )
    mask = np.asarray(dropout_mask).reshape(TOK, D_IN)
    W = np.asarray(W_base, dtype=np.float32)
    A_ = np.asarray(A, dtype=np.float32)
    B_ = np.asarray(B, dtype=np.float32)

    import ml_dtypes

    bf16 = ml_dtypes.bfloat16
    # Shared, pre-tiled weight layouts (contiguous per device DMA):
    #   WT[oc, k, p, o] = W[oc*OB+o, k*P+p]
    WT = np.ascontiguousarray(
        W.reshape(OCB, OB, KT, P).transpose(0, 2, 3, 1).astype(bf16)
    )
    #   W012[k, p, ob] = W[ob, k*P+p] for the P_OC prologue chunks
    W012 = np.ascontiguousarray(W.T[:, : P_OC * OB].astype(bf16).reshape(KT, P, P_OC * OB))
    #   AT[p, k, r] = A[r, k*P+p] / (1-p)
    AT = np.ascontiguousarray(
        (A_.T.reshape(KT, P, R).transpose(1, 0, 2) * np.float32(1.0 / (1.0 - DROP_P))).astype(bf16)
    )
    #   BT[r, o] = B[o, r] * scaling
    BT = np.ascontiguousarray((B_.T * np.float32(SCALING)).astype(bf16))

    in_maps = []
    for c in range(NCORES):
        sl = slice(c * T, (c + 1) * T)
        #   xT[k, p, t] = x[c*T + t, k*P+p]
        xT = np.ascontiguousarray(hs[sl].T.astype(bf16)).reshape(KT, P, T)
        #   mT[k, p, t] = mask[c*T + t, k*P+p] (bf16 0/1: DVE 16-bit fast path)
        mT = np.ascontiguousarray(mask[sl].T.astype(bf16)).reshape(KT, P, T)
        in_maps.append(
            {"xT": xT, "mT": mT, "WT": WT, "W012": W012, "AT": AT, "BT": BT}
        )

    nc = _get_program()
    res = run_bass_kernel_spmd(nc, in_maps, core_ids=list(range(NCORES)))
    _PROGRAM_CACHE["last_results"] = res

    # out_dev[oc, p_o, t] = out[o = oc*OB + p_o, t]  (per core)
    parts = []
    for c in range(NCORES):
        od = res.results[c]["out"]  # [OCB, P, T]
        parts.append(np.ascontiguousarray(od.reshape(D_OUT, T).T))
    out = np.concatenate(parts, axis=0)
    return out.reshape(BATCH, SEQ, D_OUT).astype(np.float32)


# revision 72
# speedup vs baseline: 1.0072x; 1.0072x over previous
"""LoRA Linear (residual + low-rank path with dropout) on 8 Trainium2 cores.

Math (fp32 reference):
  residual = hidden_states @ W_base.T
  dropped  = hidden_states * dropout_mask / (1 - p)
  out      = residual + ((dropped @ A.T) @ B.T) * scaling

Sharding: data-parallel over the 8192 tokens (8 cores x 1024 tokens);
W_base / A / B replicated. Inputs are cast to bf16 on the host (PSUM
accumulation stays fp32): same 1 cycle/row PE rate as f32r but half
the DMA bytes and SBUF, and bf16 weights get Fast-Weight-Load (4x
faster LDWEIGHTS, which fp32 is excluded from). Error ~3e-3 rel-rms /
~4e-3 scale-relative absmax over the K=4096 contraction, an order of
magnitude inside the 2e-2 gate.

Layout: W is the STATIONARY operand ([128 d, 128 o] chunks) and x is
the MOVING operand (512 tokens/matmul = one fp32 PSUM bank). One
(FWL) weight load serves the 1024-row matmul pair, so the steady
state issues back-to-back 512-row matmuls at the ~216ns hardware
floor (512 cycles @ 2.4 GHz + NX dispatch).

Schedule: the x+mask load is the serial head, so the k-loop that
computes the LoRA xa product also carries the main-matmul
accumulation for the first P_OC=3 out-chunks (6 PSUM banks + 2 xa
banks = all 8). The remaining 29 chunks then run back-to-back with W
(1 MiB/chunk) double-buffered against compute.

  - W streams exactly once (x stays resident in SBUF); host pre-tiles
    everything into large contiguous DMA runs. The P_OC prologue W
    chunks ship as one merged tensor (W012): sync-engine DMA-issue
    slots (~1.4us each), not bytes, are the scarce prologue resource,
    so x/W ride the sync ring and mask/AT ride the scalar ring.
  - Output DMAs issue from the ACT engine so the SP engine's HWDGE
    stream (all input loads) never blocks on a compute semaphore.
  - The rank-16 LoRA product accumulates into the same PSUM tile as
    the residual matmul (K=16 matmul, start=False), so the add is free.
  - 1/(1-p) is folded into A, `scaling` into B on the host.
  - Output leaves the device transposed ([out_chunk, out_lane, token]);
    the host gather undoes it.
"""

import numpy as np

P = 128
D_IN = 4096
D_OUT = 4096
BATCH, SEQ = 4, 2048
TOK = BATCH * SEQ  # 8192
NCORES = 8
T = TOK // NCORES  # 1024 tokens per core, all resident
KT = D_IN // P  # 32 k-tiles
OB = 128  # out-dim chunk width (stationary operand)
OCB = D_OUT // OB  # 32 out chunks
NT = 512  # moving free dim (tokens per matmul) = fp32 max = 1 PSUM bank
TH = T // NT  # 2 token halves
R = 16
PIECE = 4  # k-tiles per steady W DMA piece
# small leading pieces for an early PE start, coarse later blocks to
# keep the sync engine's per-DMA issue cost (~1.4us) off the critical path
BLOCKS = [(0, 1), (1, 2), (2, 4)] + [(k, k + 4) for k in range(4, KT, 4)]
P_OC = 3  # out-chunks folded into the prologue k-loop
WPRE = 1  # W prefetch depth (chunks ahead) in the steady loop
DROP_P = 0.05
SCALING = 32.0 / 16.0

_PROGRAM_CACHE = {}


def _build_program(fold_lora=False):
    """Build the device program.

    fold_lora=False: general path — dropout mask applied on-device, LoRA
    computed as two extra matmul stages (xa prologue + rank-16 accumulate).
    fold_lora=True: all-ones-mask fast path — host folds the LoRA product
    into W (W_eff = W + s/(1-p) * B@A), leaving one dense matmul; the two
    freed PSUM banks let the prologue carry 4 out-chunks instead of 3.
    """
    from concourse import bacc
    import concourse.mybir as mybir
    import concourse.tile as tile

    f32 = mybir.dt.float32
    bf16 = mybir.dt.bfloat16

    p_oc = P_OC + 1 if fold_lora else P_OC

    nc = bacc.Bacc("TRN2", target_bir_lowering=False)
    xT_d = nc.dram_tensor("xT", [KT, P, T], bf16, kind="ExternalInput")
    WT_d = nc.dram_tensor("WT", [OCB, KT, P, OB], bf16, kind="ExternalInput")
    W012_d = nc.dram_tensor("W012", [KT, P, p_oc * OB], bf16, kind="ExternalInput")
    if not fold_lora:
        mT_d = nc.dram_tensor("mT", [KT, P, T], bf16, kind="ExternalInput")
        AT_d = nc.dram_tensor("AT", [P, KT, R], bf16, kind="ExternalInput")
        BT_d = nc.dram_tensor("BT", [R, D_OUT], bf16, kind="ExternalInput")
    out_d = nc.dram_tensor("out", [OCB, P, T], f32, kind="ExternalOutput")

    with tile.TileContext(nc) as tc:
        with (
            tc.tile_pool(name="xt", bufs=1) as xtpool,
            tc.tile_pool(name="at", bufs=1) as atpool,
            tc.tile_pool(name="bt", bufs=4) as btpool,
            tc.tile_pool(name="wt", bufs=4) as wtpool,
            tc.tile_pool(name="w012", bufs=1) as w012pool,
            tc.tile_pool(name="m", bufs=2) as mpool,
            tc.tile_pool(name="d", bufs=3) as dpool,
            tc.tile_pool(name="xa", bufs=1) as xapool,
            tc.tile_pool(name="o", bufs=4) as opool,
            tc.tile_pool(name="ps_mm", bufs=8, space="PSUM") as ps_mm,
        ):
            xT_t = xtpool.tile([P, KT, T], bf16, tag="xT")
            wt = {}
            bt = {}

            def load_bt(oc):
                bt[oc] = btpool.tile([R, OB], bf16, tag="BT", name=f"BT{oc}")
                nc.sync.dma_start(bt[oc][:], BT_d[:, oc * OB : (oc + 1) * OB])

            def new_wt(oc):
                wt[oc] = wtpool.tile([P, KT, OB], bf16, tag="WT", name=f"WT{oc}")

            def load_wt_piece(oc, k0, n=PIECE):
                nc.sync.dma_start(
                    wt[oc][:, k0 : k0 + n],
                    WT_d[oc, k0 : k0 + n].rearrange("k p o -> p k o"),
                )

            def load_wt(oc):
                new_wt(oc)
                for k0 in range(0, KT, 2 * PIECE):
                    load_wt_piece(oc, k0, 2 * PIECE)

            # the prologue W chunks share one tile so each k-block is a
            # single merged DMA (sync-engine issue slots are the scarce
            # resource in the prologue, not bytes)
            w012 = w012pool.tile([P, KT, p_oc, OB], bf16, tag="W012", name="W012t")
            if not fold_lora:
                at_t = atpool.tile([P, KT, R], bf16, tag="AT")
                for o in range(p_oc):
                    load_bt(o)
                xa_ps = [
                    ps_mm.tile([R, NT], f32, tag="ps", name=f"xa_ps{h}")
                    for h in range(TH)
                ]
            pro_ps = {
                (o, h): ps_mm.tile([P, NT], f32, tag="ps", name=f"pps{o}_{h}")
                for o in range(p_oc)
                for h in range(TH)
            }

            # ---- PE warm-up: the HAM clock gate holds the PE at 1.2 GHz
            # until ~3.4us of sustained activity. Burn that window on dummy
            # matmuls over memset data while the first DMA pieces are still
            # in flight; the real start=True matmul overwrites the bank.
            dm_t = dpool.tile([P, NT], bf16, tag="d", name="dwarm")
            nc.vector.memset(dm_t[:], 0.0)
            warm_ps = xa_ps[0] if not fold_lora else pro_ps[0, 0]
            for _ in range(10):
                nc.tensor.matmul(
                    warm_ps[:, :NT] if fold_lora else warm_ps[:],
                    dm_t[:, : (P if fold_lora else R)],
                    dm_t[:],
                    start=True,
                    stop=True,
                )

            # ---- prologue k-loop: x (and mask) stream in; the first p_oc
            # out-chunks of the residual matmul accumulate (plus the LoRA
            # xa product in the general path).
            m_t = None
            mk0 = 0
            for kb, (k0, k1) in enumerate(BLOCKS):
                n = k1 - k0
                # sync engine: x then the merged W block; scalar engine
                # (idle until the first drain) issues mask/AT in parallel
                nc.sync.dma_start(
                    xT_t[:, k0:k1],
                    xT_d[k0:k1].rearrange("k p t -> p k t"),
                )
                nc.sync.dma_start(
                    w012[:, k0:k1],
                    W012_d[k0:k1].rearrange("k p b -> p k b"),
                )
                if not fold_lora:
                    m_t = mpool.tile([P, n, T], bf16, tag="m", name=f"m{k0}")
                    mk0 = k0
                    nc.scalar.dma_start(
                        m_t[:], mT_d[k0:k1].rearrange("k p t -> p k t")
                    )
                    if kb == 0:
                        nc.scalar.dma_start(at_t[:], AT_d[:])
                for k in range(k0, k1):
                    dts = []
                    if not fold_lora:
                        for h in range(TH):
                            hs = slice(h * NT, (h + 1) * NT)
                            d_t = dpool.tile(
                                [P, NT], bf16, tag="d", name=f"d{k}_{h}"
                            )
                            nc.vector.tensor_tensor(
                                d_t[:],
                                xT_t[:, k, hs],
                                m_t[:, k - mk0, hs],
                                mybir.AluOpType.mult,
                            )
                            dts.append(d_t)
                    for o in range(p_oc):
                        for h in range(TH):
                            hs = slice(h * NT, (h + 1) * NT)
                            nc.tensor.matmul(
                                pro_ps[o, h][:],
                                w012[:, k, o],
                                xT_t[:, k, hs],
                                start=(k == 0),
                                stop=(fold_lora and k == KT - 1),
                            )
                    if not fold_lora:
                        for h in range(TH):
                            nc.tensor.matmul(
                                xa_ps[h][:],
                                at_t[:, k],
                                dts[h][:],
                                start=(k == 0),
                                stop=(k == KT - 1),
                            )
            # W prefetch for the first steady chunk: the sync ring drains
            # ~20us before the PE finishes the prologue, so this lands early.
            load_wt(p_oc)

            if not fold_lora:
                xaT_t = xapool.tile([R, T], bf16, tag="xaT")
                for h in range(TH):
                    nc.vector.tensor_copy(
                        xaT_t[:, h * NT : (h + 1) * NT], xa_ps[h][:]
                    )

            def finish(oc, pss):
                # rank-16 LoRA accumulate (general path only) + drain
                if not fold_lora:
                    for h in range(TH):
                        hs = slice(h * NT, (h + 1) * NT)
                        nc.tensor.matmul(
                            pss[h][:],
                            bt[oc][:],
                            xaT_t[:, hs],
                            start=False,
                            stop=True,
                        )
                for h in range(TH):
                    hs = slice(h * NT, (h + 1) * NT)
                    o_t = opool.tile([P, NT], f32, tag="o", name=f"o{oc}_{h}")
                    nc.vector.tensor_copy(o_t[:], pss[h][:])
                    nc.scalar.dma_start(out_d[oc, :, hs], o_t[:])

            for o in range(p_oc):
                finish(o, [pro_ps[o, h] for h in range(TH)])

            # ---- steady loop over the remaining out-chunks
            for oc in range(p_oc, OCB):
                if not fold_lora:
                    load_bt(oc)
                if oc + WPRE < OCB:
                    load_wt(oc + WPRE)
                pss = [
                    ps_mm.tile([P, NT], f32, tag="ps", name=f"ps{oc}_{h}")
                    for h in range(TH)
                ]
                for k in range(KT):
                    for h in range(TH):
                        nc.tensor.matmul(
                            pss[h][:],
                            wt[oc][:, k],
                            xT_t[:, k, h * NT : (h + 1) * NT],
                            start=(k == 0),
                            stop=(fold_lora and k == KT - 1),
                        )
                finish(oc, pss)
                del wt[oc]

    nc.finalize()
    return nc


def _get_program(fold_lora=False):
    key = ("fold" if fold_lora else "general")
    if key not in _PROGRAM_CACHE:
        _PROGRAM_CACHE[key] = _build_program(fold_lora)
    return _PROGRAM_CACHE[key]


def kernel(hidden_states, W_base, A, B, dropout_mask):
    from concourse.bass_utils import run_bass_kernel_spmd

    hs = np.ascontiguousarray(np.asarray(hidden_states, dtype=np.float32)).reshape(
        TOK, D_IN
    )
    mask = np.asarray(dropout_mask).reshape(TOK, D_IN)
    W = np.asarray(W_base, dtype=np.float32)
    A_ = np.asarray(A, dtype=np.float32)
    B_ = np.asarray(B, dtype=np.float32)

    import ml_dtypes

    bf16 = ml_dtypes.bfloat16

    # All-ones mask (the dropout no-op case): fold the LoRA product into W
    # on the host and run the pure-matmul program.
    fold = bool(mask.all())
    if fold:
        W = W + np.float32(SCALING / (1.0 - DROP_P)) * (B_ @ A_)
    p_oc = P_OC + 1 if fold else P_OC

    # Shared, pre-tiled weight layouts (contiguous per device DMA):
    #   WT[oc, k, p, o] = W[oc*OB+o, k*P+p]
    WT = np.ascontiguousarray(
        W.reshape(OCB, OB, KT, P).transpose(0, 2, 3, 1).astype(bf16)
    )
    #   W012[k, p, ob] = W[ob, k*P+p] for the prologue chunks
    W012 = np.ascontiguousarray(
        W.T[:, : p_oc * OB].astype(bf16).reshape(KT, P, p_oc * OB)
    )
    if not fold:
        #   AT[p, k, r] = A[r, k*P+p] / (1-p)
        AT = np.ascontiguousarray(
            (
                A_.T.reshape(KT, P, R).transpose(1, 0, 2)
                * np.float32(1.0 / (1.0 - DROP_P))
            ).astype(bf16)
        )
        #   BT[r, o] = B[o, r] * scaling
        BT = np.ascontiguousarray((B_.T * np.float32(SCALING)).astype(bf16))

    in_maps = []
    for c in range(NCORES):
        sl = slice(c * T, (c + 1) * T)
        #   xT[k, p, t] = x[c*T + t, k*P+p]
        xT = np.ascontiguousarray(hs[sl].T.astype(bf16)).reshape(KT, P, T)
        im = {"xT": xT, "WT": WT, "W012": W012}
        if not fold:
            #   mT[k, p, t] = mask[c*T + t, k*P+p] (bf16: DVE 16-bit path)
            im["mT"] = np.ascontiguousarray(mask[sl].T.astype(bf16)).reshape(
                KT, P, T
            )
            im["AT"] = AT
            im["BT"] = BT
        in_maps.append(im)

    nc = _get_program(fold)
    res = run_bass_kernel_spmd(nc, in_maps, core_ids=list(range(NCORES)))
    _PROGRAM_CACHE["last_results"] = res

    # out_dev[oc, p_o, t] = out[o = oc*OB + p_o, t]  (per core)
    parts = []
    for c in range(NCORES):
        od = res.results[c]["out"]  # [OCB, P, T]
        parts.append(np.ascontiguousarray(od.reshape(D_OUT, T).T))
    out = np.concatenate(parts, axis=0)
    return out.reshape(BATCH, SEQ, D_OUT).astype(np.float32)
